# revision 12
# baseline (speedup 1.0000x reference)
import numpy as np

B, N, D, H, DK, M = 2, 2048, 1024, 16, 64, 64
NJ = N + M
JT = 17
CPC = 256
FQ = 1024
MASK_BIAS = -240000.0


def build_kernel(loop_n=None):
    from contextlib import ExitStack
    import concourse.bass as bass
    import concourse.tile as tile
    import concourse.mybir as mybir
    from concourse import bacc

    f32 = mybir.dt.float32
    bf16 = mybir.dt.bfloat16
    EXP = mybir.ActivationFunctionType.Exp

    nc = bacc.Bacc(None, target_bir_lowering=False, debug=False)

    qT = nc.declare_dram_parameter("qT", [D, N], f32, isOutput=False)
    kT = nc.declare_dram_parameter("kT", [D, N], f32, isOutput=False)
    vT = nc.declare_dram_parameter("vT", [D, N], f32, isOutput=False)
    wqT = nc.declare_dram_parameter("wqT", [128, 8, CPC], f32, isOutput=False)
    wkT = nc.declare_dram_parameter("wkT", [128, 8, CPC], f32, isOutput=False)
    wvT = nc.declare_dram_parameter("wvT", [128, 8, CPC], f32, isOutput=False)
    woT = nc.declare_dram_parameter("woT", [128, 2, D], f32, isOutput=False)
    mkT = nc.declare_dram_parameter("mkT", [CPC, M], f32, isOutput=False)
    mv = nc.declare_dram_parameter("mv", [M, CPC], f32, isOutput=False)
    bqT = nc.declare_dram_parameter("bqT", [128, 2], f32, isOutput=False)
    bkT = nc.declare_dram_parameter("bkT", [128, 2], f32, isOutput=False)
    bv = nc.declare_dram_parameter("bv", [1, CPC], f32, isOutput=False)
    mb = nc.declare_dram_parameter("mb", [128, JT], f32, isOutput=False)
    out = nc.declare_dram_parameter("out", [N, D], f32, isOutput=True)

    with tile.TileContext(nc) as tc, ExitStack() as ctx:
        if loop_n is not None:
            ctx.enter_context(tc.For_i(0, loop_n, 1))
        consts = ctx.enter_context(tc.tile_pool(name="consts", bufs=1))
        wpool = ctx.enter_context(tc.tile_pool(name="wpool", bufs=1))
        proj = ctx.enter_context(tc.tile_pool(name="proj", bufs=1))
        ptp = ctx.enter_context(tc.tile_pool(name="ptp", bufs=20))
        onrm = ctx.enter_context(tc.tile_pool(name="onrm", bufs=1))
        small = ctx.enter_context(tc.tile_pool(name="small", bufs=1))
        ysb = ctx.enter_context(tc.tile_pool(name="ysb", bufs=2))
        ps = ctx.enter_context(tc.tile_pool(name="ps", bufs=2, space="PSUM"))
        po = ctx.enter_context(tc.tile_pool(name="po", bufs=2, space="PSUM"))
        dsc = ctx.enter_context(tc.tile_pool(name="dsc", bufs=2, space="DRAM"))

        ones128 = consts.tile([128, 128], f32, tag="ones")
        nc.vector.memset(ones128, 1.0)
        mb_sb = consts.tile([128, JT], f32, tag="mb")
        nc.sync.dma_start(out=mb_sb, in_=mb[:])
        bq_sb = consts.tile([128, 2], f32, tag="bq")
        nc.sync.dma_start(out=bq_sb, in_=bqT[:])
        bk_sb = consts.tile([128, 2], f32, tag="bk")
        nc.sync.dma_start(out=bk_sb, in_=bkT[:])
        bv_sb = consts.tile([1, CPC], f32, tag="bv")
        nc.sync.dma_start(out=bv_sb, in_=bv[:])
        bvb_ps = ps.tile([128, CPC], f32, tag="s")
        nc.tensor.matmul(bvb_ps[:, :], ones128[0:1, :], bv_sb[:, :],
                         start=True, stop=True)
        bvb_sb = consts.tile([128, 4, DK], f32, tag="bvb")
        nc.vector.tensor_copy(
            bvb_sb[:, :, :], bvb_ps.rearrange("p (h d) -> p h d", h=4))

        qh_sb = [proj.tile([128, N], bf16, tag=f"qh{p}", name=f"qh{p}")
                 for p in range(2)]
        kh_sb = [proj.tile([128, NJ], bf16, tag=f"kh{p}", name=f"kh{p}")
                 for p in range(2)]
        vt_sb = [proj.tile([128, 4, DK + 1], bf16, tag=f"vt{j}",
                           name=f"vt{j}") for j in range(JT)]
        on_sb = [onrm.tile([128, N], bf16, tag=f"on{p}", name=f"on{p}")
                 for p in range(2)]

        inbf = ctx.enter_context(tc.tile_pool(name="inbf", bufs=2))
        stage = ctx.enter_context(tc.tile_pool(name="stage", bufs=2))

        def load_weight(dram, tag):
            wf = stage.tile([128, 8, CPC], f32, tag="wstage",
                            name=f"wf_{tag}")
            nc.sync.dma_start(out=wf, in_=dram[:])
            wb = wpool.tile([128, 8, CPC], bf16, tag=tag, name=f"wb_{tag}")
            nc.vector.tensor_copy(wb[:, :, :], wf[:, :, :])
            return wb

        def load_half(dram, nh, nm):
            xb = inbf.tile([128, 8, FQ], bf16, tag="xbf", name=f"xbf_{nm}{nh}")
            for dc in range(8):
                xf = stage.tile([128, FQ], f32, tag="xstage",
                                name=f"xf_{nm}{nh}{dc}", bufs=3)
                eng = nc.sync if dc % 2 == 0 else nc.gpsimd
                eng.dma_start(
                    out=xf,
                    in_=dram[dc * 128:(dc + 1) * 128, nh * FQ:(nh + 1) * FQ])
                nc.vector.tensor_copy(xb[:, dc, :], xf[:, :])
            return xb

        def proj_half(wb, x_bf, out_sb, b_sb, nh, nm):
            for p in range(2):
                for qc in range(2):
                    pp = ps.tile([128, 512], f32, tag="s",
                                 name=f"pp_{nm}{nh}{p}{qc}")
                    for dc in range(8):
                        nc.tensor.matmul(
                            pp[:, :],
                            wb[:, dc, p * 128:(p + 1) * 128],
                            x_bf[:, dc, qc * 512:(qc + 1) * 512],
                            start=(dc == 0), stop=(dc == 7))
                    o0 = nh * FQ + qc * 512
                    nc.vector.tensor_scalar_add(
                        out_sb[p][:, o0:o0 + 512], pp[:, :], b_sb[:, p:p + 1])

        def vproj_half(wv_bf, v_bf, nh):
            for t in range(8):
                jt = nh * 8 + t
                pp = ps.tile([128, CPC], f32, tag="s", name=f"ppv{jt}")
                for dc in range(8):
                    nc.tensor.matmul(
                        pp[:, :],
                        v_bf[:, dc, t * 128:(t + 1) * 128],
                        wv_bf[:, dc, :],
                        start=(dc == 0), stop=(dc == 7))
                vt = vt_sb[jt]
                nc.vector.tensor_add(
                    vt[:, :, 0:DK],
                    pp.rearrange("p (h d) -> p h d", h=4),
                    bvb_sb[:, :, :])
                nc.vector.memset(vt[:, :, DK:DK + 1], 1.0)

        wq_bf = load_weight(wqT, "wq")
        wk_bf = load_weight(wkT, "wk")
        wv_bf = load_weight(wvT, "wv")
        wo_f = stage.tile([128, 2, D], f32, tag="wstage", name="wo_f")
        nc.sync.dma_start(out=wo_f, in_=woT[:])
        wo_bf = wpool.tile([128, 2, D], bf16, tag="wo", name="wo_bf")
        nc.vector.tensor_copy(wo_bf[:, :, :], wo_f[:, :, :])

        k_bf0 = load_half(kT, 0, "k")
        proj_half(wk_bf, k_bf0, kh_sb, bk_sb, 0, "k")
        k_bf1 = load_half(kT, 1, "k")
        proj_half(wk_bf, k_bf1, kh_sb, bk_sb, 1, "k")
        for p in range(2):
            mkf = stage.tile([128, M], f32, tag="xstage", bufs=3,
                             name=f"mkf{p}")
            nc.sync.dma_start(out=mkf, in_=mkT[p * 128:(p + 1) * 128, :])
            nc.vector.tensor_copy(kh_sb[p][:, N:NJ], mkf[:, :])

        q_bf0 = load_half(qT, 0, "q")
        proj_half(wq_bf, q_bf0, qh_sb, bq_sb, 0, "q")

        v_bf0 = load_half(vT, 0, "v")
        vproj_half(wv_bf, v_bf0, 0)
        v_bf1 = load_half(vT, 1, "v")
        vproj_half(wv_bf, v_bf1, 1)
        mvf = stage.tile([M, CPC], f32, tag="xstage", bufs=3, name="mvf")
        nc.sync.dma_start(out=mvf, in_=mv[:])
        vt = vt_sb[16]
        nc.vector.tensor_copy(
            vt[0:M, :, 0:DK], mvf.rearrange("p (h d) -> p h d", h=4))
        nc.vector.memset(vt[0:M, :, DK:DK + 1], 1.0)

        def emit_y(nt):
            yp = ps.tile([128, D], f32, tag="s", name=f"yp{nt}")
            for c in range(2):
                for cc in range(2):
                    nc.tensor.matmul(
                        yp[:, c * 512:(c + 1) * 512],
                        on_sb[cc][:, nt * 128:(nt + 1) * 128],
                        wo_bf[:, cc, c * 512:(c + 1) * 512],
                        start=(cc == 0), stop=(cc == 1))
            yt = ysb.tile([128, D], f32, tag="y", name=f"yt{nt}")
            nc.vector.tensor_copy(yt[:, :], yp[:, :])
            nc.sync.dma_start(out=out[nt * 128:(nt + 1) * 128, :], in_=yt)

        i32 = mybir.dt.int32
        C0, C1, C2 = -0.23549792, 2.0017324, 2.0

        def attention(qh, p):
            q0 = qh * FQ
            pt = {}
            for jt in range(JT):
                jw = 128 if jt < 16 else M
                for s in range(2):
                    sp = ps.tile([128, FQ], f32, tag="s",
                                 name=f"sp{qh}{p}{jt}{s}")
                    for c in range(2):
                        nc.tensor.matmul(
                            sp[:jw, c * 512:(c + 1) * 512],
                            kh_sb[p][s * 64:s * 64 + 64,
                                     jt * 128:jt * 128 + jw],
                            qh_sb[p][s * 64:s * 64 + 64,
                                     q0 + c * 512:q0 + (c + 1) * 512],
                            start=True, stop=True)
                    pe = ptp.tile([128, FQ], bf16, tag="pt",
                                  name=f"pt{qh}{p}{jt}{s}")
                    nc.scalar.activation(
                        out=pe[:jw, :], in_=sp[:jw, :], func=EXP,
                        bias=mb_sb[:jw, jt:jt + 1], scale=0.125)
                    pt[(s, jt)] = pe
                if qh == 1 and p == 0 and jt % 2 == 1:
                    emit_y(jt // 2)
            ops = []
            for s in range(2):
                op = po.tile([DK + 1, FQ], f32, tag="o", name=f"op{qh}{p}{s}")
                for jt in range(JT):
                    jw = 128 if jt < 16 else M
                    for c in range(2):
                        nc.tensor.matmul(
                            op[:, c * 512:(c + 1) * 512],
                            vt_sb[jt][:jw, 2 * p + s, :],
                            pt[(s, jt)][:jw, c * 512:(c + 1) * 512],
                            start=(jt == 0), stop=(jt == JT - 1))
                ops.append(op)
            dpk = small.tile([33, FQ], f32, tag="dpk", name=f"dpk{qh}{p}")
            nc.vector.memset(dpk[:, :], 1.0)
            nc.vector.tensor_copy(dpk[0:1, :], ops[0][DK:DK + 1, :])
            nc.vector.tensor_copy(dpk[32:33, :], ops[1][DK:DK + 1, :])
            nx = small.tile([33, FQ], f32, tag="nx", name=f"nx{qh}{p}")
            nc.vector.tensor_scalar(
                out=nx.bitcast(i32)[:, :], in0=dpk.bitcast(i32)[:, :],
                scalar1=-1, scalar2=None, op0=mybir.AluOpType.bitwise_xor)
            ya = small.tile([33, FQ], f32, tag="ya", name=f"ya{qh}{p}")
            nc.vector.tensor_scalar_mul(ya[:, :], nx[:, :], C0)
            tt = small.tile([33, FQ], f32, tag="tt", name=f"tt{qh}{p}")
            nc.vector.tensor_mul(tt[:, :], dpk[:, :], ya[:, :])
            nc.vector.tensor_scalar(
                out=tt[:, :], in0=tt[:, :], scalar1=-1.0, scalar2=C1,
                op0=mybir.AluOpType.mult, op1=mybir.AluOpType.add)
            yb = small.tile([33, FQ], f32, tag="yb", name=f"yb{qh}{p}")
            nc.vector.tensor_mul(yb[:, :], ya[:, :], tt[:, :])
            nc.vector.tensor_mul(tt[:, :], dpk[:, :], yb[:, :])
            nc.vector.tensor_scalar(
                out=tt[:, :], in0=tt[:, :], scalar1=-1.0, scalar2=C2,
                op0=mybir.AluOpType.mult, op1=mybir.AluOpType.add)
            nc.vector.tensor_mul(ya[:, :], yb[:, :], tt[:, :])
            dr = dsc.tile([2, FQ], f32, tag="dr", name=f"dr{qh}{p}")
            for s in range(2):
                nc.sync.dma_start(out=dr[s:s + 1, :], in_=ya[32 * s:32 * s + 1, :])
            for s in range(2):
                rbc = small.tile([DK, FQ], f32, tag=f"rbc{s}",
                                 name=f"rbc{qh}{p}{s}")
                row = dr[s:s + 1, :]
                bcast_src = bass.AP(
                    tensor=row.tensor, offset=row.offset,
                    ap=[[0, DK]] + [list(x) for x in row.ap[1:]])
                nc.sync.dma_start(out=rbc, in_=bcast_src)
                nc.vector.tensor_mul(
                    on_sb[p][s * 64:s * 64 + 64, q0:q0 + FQ],
                    ops[s][0:DK, :], rbc[:, :])

        attention(0, 0)
        attention(0, 1)
        q_bf1 = load_half(qT, 1, "q")
        proj_half(wq_bf, q_bf1, qh_sb, bq_sb, 1, "q")
        attention(1, 0)
        attention(1, 1)
        for nt in range(8, 16):
            emit_y(nt)

    nc.compile()
    return nc


_CACHED_NC = None


def _in_maps(q, k, v, attention_mask, Wq, bq, Wk, bk, Wv, bv, Wo, bo,
             m_k, m_v):
    def tile_w(wT_slice):
        C = wT_slice.shape[1]
        return np.ascontiguousarray(
            wT_slice.reshape(8, 128, C).transpose(1, 0, 2))

    mk_s = np.asarray(m_k[0], np.float32) * float(np.sqrt(DK))
    mv_s = np.asarray(m_v[0], np.float32) * float(np.sqrt(M))
    WqT = np.asarray(Wq, np.float32).T
    WkT = np.asarray(Wk, np.float32).T
    WvT = np.asarray(Wv, np.float32).T
    WoT = np.asarray(Wo, np.float32).T
    maps = []
    for core in range(8):
        b, part = divmod(core, 4)
        c0 = part * CPC
        msk = np.asarray(attention_mask[b, 0, 0])
        mbias = np.zeros(JT * 128, np.float32)
        mbias[:N] = np.where(msk, MASK_BIAS, 0.0)
        maps.append({
            "qT": np.ascontiguousarray(np.asarray(q[b], np.float32).T),
            "kT": np.ascontiguousarray(np.asarray(k[b], np.float32).T),
            "vT": np.ascontiguousarray(np.asarray(v[b], np.float32).T),
            "wqT": tile_w(WqT[:, c0:c0 + CPC]),
            "wkT": tile_w(WkT[:, c0:c0 + CPC]),
            "wvT": tile_w(WvT[:, c0:c0 + CPC]),
            "woT": np.ascontiguousarray(
                WoT[c0:c0 + CPC, :].reshape(2, 128, D).transpose(1, 0, 2)),
            "mkT": np.ascontiguousarray(mk_s.T[c0:c0 + CPC, :]),
            "mv": np.ascontiguousarray(mv_s[:, c0:c0 + CPC]),
            "bqT": np.ascontiguousarray(
                np.asarray(bq, np.float32)[c0:c0 + CPC].reshape(2, 128).T),
            "bkT": np.ascontiguousarray(
                np.asarray(bk, np.float32)[c0:c0 + CPC].reshape(2, 128).T),
            "bv": np.ascontiguousarray(
                np.asarray(bv, np.float32)[c0:c0 + CPC].reshape(1, CPC)),
            "mb": np.ascontiguousarray(mbias.reshape(JT, 128).T),
        })
    return maps


def kernel(q, k, v, attention_mask, Wq, bq, Wk, bk, Wv, bv, Wo, bo,
           m_k, m_v):
    from concourse.bass_utils import run_bass_kernel_spmd
    global _CACHED_NC
    if _CACHED_NC is None:
        _CACHED_NC = build_kernel()
    maps = _in_maps(q, k, v, attention_mask, Wq, bq, Wk, bk, Wv, bv,
                    Wo, bo, m_k, m_v)
    res = run_bass_kernel_spmd(_CACHED_NC, maps, core_ids=list(range(8)))
    out = np.zeros((B, N, D), np.float32)
    for core in range(8):
        out[core // 4] += res.results[core]["out"]
    out += np.asarray(bo, np.float32)[None, None, :]
    return out


# revision 15
# speedup vs baseline: 2.0056x; 2.0056x over previous
import numpy as np

B, N, D, H, DK, M = 2, 2048, 1024, 16, 64, 64
NJ = N + M
JT = 17
CPC = 256
FQ = 1024
MASK_BIAS = -240000.0


def build_kernel(loop_n=None):
    from contextlib import ExitStack
    import concourse.bass as bass
    import concourse.tile as tile
    import concourse.mybir as mybir
    from concourse import bacc

    f32 = mybir.dt.float32
    bf16 = mybir.dt.bfloat16
    EXP = mybir.ActivationFunctionType.Exp

    nc = bacc.Bacc(None, target_bir_lowering=False, debug=False)

    qT = nc.declare_dram_parameter("qT", [2, D, FQ], f32, isOutput=False)
    kT = nc.declare_dram_parameter("kT", [2, D, FQ], f32, isOutput=False)
    vT = nc.declare_dram_parameter("vT", [2, D, FQ], f32, isOutput=False)
    wqT = nc.declare_dram_parameter("wqT", [128, 8, CPC], f32, isOutput=False)
    wkT = nc.declare_dram_parameter("wkT", [128, 8, CPC], f32, isOutput=False)
    wvT = nc.declare_dram_parameter("wvT", [128, 8, CPC], f32, isOutput=False)
    woT = nc.declare_dram_parameter("woT", [128, 2, D], f32, isOutput=False)
    mkT = nc.declare_dram_parameter("mkT", [CPC, M], f32, isOutput=False)
    mv = nc.declare_dram_parameter("mv", [M, CPC], f32, isOutput=False)
    bqT = nc.declare_dram_parameter("bqT", [128, 2], f32, isOutput=False)
    bkT = nc.declare_dram_parameter("bkT", [128, 2], f32, isOutput=False)
    bv = nc.declare_dram_parameter("bv", [1, CPC], f32, isOutput=False)
    mb = nc.declare_dram_parameter("mb", [128, JT], f32, isOutput=False)
    out = nc.declare_dram_parameter("out", [N, D], f32, isOutput=True)

    with tile.TileContext(nc) as tc, ExitStack() as ctx:
        if loop_n is not None:
            ctx.enter_context(tc.For_i(0, loop_n, 1))
        consts = ctx.enter_context(tc.tile_pool(name="consts", bufs=1))
        wpool = ctx.enter_context(tc.tile_pool(name="wpool", bufs=1))
        proj = ctx.enter_context(tc.tile_pool(name="proj", bufs=1))
        onrm = ctx.enter_context(tc.tile_pool(name="onrm", bufs=1))
        small = ctx.enter_context(tc.tile_pool(name="small", bufs=1))
        ysb = ctx.enter_context(tc.tile_pool(name="ysb", bufs=2))
        ps = ctx.enter_context(tc.tile_pool(name="ps", bufs=2, space="PSUM"))
        po = ctx.enter_context(tc.tile_pool(name="po", bufs=2, space="PSUM"))

        ones128 = consts.tile([128, 128], f32, tag="ones")
        nc.vector.memset(ones128, 1.0)
        mb_sb = consts.tile([128, JT], f32, tag="mb")
        nc.sync.dma_start(out=mb_sb, in_=mb[:])
        bq_sb = consts.tile([128, 2], f32, tag="bq")
        nc.sync.dma_start(out=bq_sb, in_=bqT[:])
        bk_sb = consts.tile([128, 2], f32, tag="bk")
        nc.sync.dma_start(out=bk_sb, in_=bkT[:])
        bv_sb = consts.tile([1, CPC], f32, tag="bv")
        nc.sync.dma_start(out=bv_sb, in_=bv[:])
        bvb_ps = ps.tile([128, CPC], f32, tag="s")
        nc.tensor.matmul(bvb_ps[:, :], ones128[0:1, :], bv_sb[:, :],
                         start=True, stop=True)
        bvb_sb = consts.tile([128, 4, DK], f32, tag="bvb")
        nc.vector.tensor_copy(
            bvb_sb[:, :, :], bvb_ps.rearrange("p (h d) -> p h d", h=4))

        qh_sb = [proj.tile([128, N], bf16, tag=f"qh{p}", name=f"qh{p}")
                 for p in range(2)]
        kh_sb = [proj.tile([128, NJ], bf16, tag=f"kh{p}", name=f"kh{p}")
                 for p in range(2)]
        vt_sb = [proj.tile([128, 4, DK + 1], bf16, tag=f"vt{j}",
                           name=f"vt{j}") for j in range(JT)]
        on_sb = [onrm.tile([128, N], bf16, tag=f"on{p}", name=f"on{p}")
                 for p in range(2)]

        pha = ExitStack()
        inbf = pha.enter_context(tc.tile_pool(name="inbf", bufs=2))
        stage = pha.enter_context(tc.tile_pool(name="stage", bufs=2))

        def load_weight(dram, tag):
            wf = stage.tile([128, 8, CPC], f32, tag="wstage",
                            name=f"wf_{tag}")
            nc.sync.dma_start(out=wf, in_=dram[:])
            wb = wpool.tile([128, 8, CPC], bf16, tag=tag, name=f"wb_{tag}")
            nc.vector.tensor_copy(wb[:, :, :], wf[:, :, :])
            return wb

        def load_half(dram, nh, nm):
            xb = inbf.tile([128, 8, FQ], bf16, tag="xbf", name=f"xbf_{nm}{nh}")
            for dc in range(8):
                xf = stage.tile([128, FQ], f32, tag="xstage",
                                name=f"xf_{nm}{nh}{dc}", bufs=3)
                nc.sync.dma_start(
                    out=xf, in_=dram[nh, dc * 128:(dc + 1) * 128, :])
                nc.vector.tensor_copy(xb[:, dc, :], xf[:, :])
            return xb

        def proj_half(wb, x_bf, out_sb, b_sb, nh, nm):
            for p in range(2):
                for qc in range(2):
                    pp = ps.tile([128, 512], f32, tag="s",
                                 name=f"pp_{nm}{nh}{p}{qc}")
                    for dc in range(8):
                        nc.tensor.matmul(
                            pp[:, :],
                            wb[:, dc, p * 128:(p + 1) * 128],
                            x_bf[:, dc, qc * 512:(qc + 1) * 512],
                            start=(dc == 0), stop=(dc == 7))
                    o0 = nh * FQ + qc * 512
                    nc.vector.tensor_scalar_add(
                        out_sb[p][:, o0:o0 + 512], pp[:, :], b_sb[:, p:p + 1])

        def vproj_half(wv_bf, v_bf, nh):
            for t in range(8):
                jt = nh * 8 + t
                pp = ps.tile([128, CPC], f32, tag="s", name=f"ppv{jt}")
                for dc in range(8):
                    nc.tensor.matmul(
                        pp[:, :],
                        v_bf[:, dc, t * 128:(t + 1) * 128],
                        wv_bf[:, dc, :],
                        start=(dc == 0), stop=(dc == 7))
                vt = vt_sb[jt]
                nc.vector.tensor_add(
                    vt[:, :, 0:DK],
                    pp.rearrange("p (h d) -> p h d", h=4),
                    bvb_sb[:, :, :])
                nc.vector.memset(vt[:, :, DK:DK + 1], 1.0)

        wq_bf = load_weight(wqT, "wq")
        wk_bf = load_weight(wkT, "wk")
        wv_bf = load_weight(wvT, "wv")
        wo_f = stage.tile([128, 2, D], f32, tag="wstage", name="wo_f")
        nc.sync.dma_start(out=wo_f, in_=woT[:])
        wo_bf = wpool.tile([128, 2, D], bf16, tag="wo", name="wo_bf")
        nc.vector.tensor_copy(wo_bf[:, :, :], wo_f[:, :, :])

        k_bf0 = load_half(kT, 0, "k")
        proj_half(wk_bf, k_bf0, kh_sb, bk_sb, 0, "k")
        k_bf1 = load_half(kT, 1, "k")
        proj_half(wk_bf, k_bf1, kh_sb, bk_sb, 1, "k")
        for p in range(2):
            mkf = stage.tile([128, M], f32, tag="xstage", bufs=3,
                             name=f"mkf{p}")
            nc.sync.dma_start(out=mkf, in_=mkT[p * 128:(p + 1) * 128, :])
            nc.vector.tensor_copy(kh_sb[p][:, N:NJ], mkf[:, :])

        q_bf0 = load_half(qT, 0, "q")
        proj_half(wq_bf, q_bf0, qh_sb, bq_sb, 0, "q")

        v_bf0 = load_half(vT, 0, "v")
        vproj_half(wv_bf, v_bf0, 0)
        v_bf1 = load_half(vT, 1, "v")
        vproj_half(wv_bf, v_bf1, 1)
        mvf = stage.tile([M, CPC], f32, tag="xstage", bufs=3, name="mvf")
        nc.sync.dma_start(out=mvf, in_=mv[:])
        vt = vt_sb[16]
        nc.vector.tensor_copy(
            vt[0:M, :, 0:DK], mvf.rearrange("p (h d) -> p h d", h=4))
        nc.vector.memset(vt[0:M, :, DK:DK + 1], 1.0)

        def emit_y(nt):
            yp = ps.tile([128, D], f32, tag="s", name=f"yp{nt}")
            for c in range(2):
                for cc in range(2):
                    nc.tensor.matmul(
                        yp[:, c * 512:(c + 1) * 512],
                        on_sb[cc][:, nt * 128:(nt + 1) * 128],
                        wo_bf[:, cc, c * 512:(c + 1) * 512],
                        start=(cc == 0), stop=(cc == 1))
            yt = ysb.tile([128, D], f32, tag="y", name=f"yt{nt}")
            nc.vector.tensor_copy(yt[:, :], yp[:, :])
            nc.sync.dma_start(out=out[nt * 128:(nt + 1) * 128, :], in_=yt)

        i32 = mybir.dt.int32
        C0, C1, C2 = -0.23549792, 2.0017324, 2.0

        def attention(qh, p):
            q0 = qh * FQ
            pt = {}
            for jt in range(JT):
                jw = 128 if jt < 16 else M
                for s in range(2):
                    sp = ps.tile([128, FQ], f32, tag="s",
                                 name=f"sp{qh}{p}{jt}{s}")
                    for c in range(2):
                        nc.tensor.matmul(
                            sp[:jw, c * 512:(c + 1) * 512],
                            kh_sb[p][s * 64:s * 64 + 64,
                                     jt * 128:jt * 128 + jw],
                            qh_sb[p][s * 64:s * 64 + 64,
                                     q0 + c * 512:q0 + (c + 1) * 512],
                            start=True, stop=True)
                    pe = ptp.tile([128, FQ], bf16, tag="pt",
                                  name=f"pt{qh}{p}{jt}{s}")
                    nc.scalar.activation(
                        out=pe[:jw, :], in_=sp[:jw, :], func=EXP,
                        bias=mb_sb[:jw, jt:jt + 1], scale=0.125)
                    pt[(s, jt)] = pe
                if qh == 1 and p == 0 and jt % 2 == 1:
                    emit_y(jt // 2)
            ops = []
            for s in range(2):
                op = po.tile([DK + 1, FQ], f32, tag="o", name=f"op{qh}{p}{s}")
                for jt in range(JT):
                    jw = 128 if jt < 16 else M
                    for c in range(2):
                        nc.tensor.matmul(
                            op[:, c * 512:(c + 1) * 512],
                            vt_sb[jt][:jw, 2 * p + s, :],
                            pt[(s, jt)][:jw, c * 512:(c + 1) * 512],
                            start=(jt == 0), stop=(jt == JT - 1))
                ops.append(op)
            dpk = small.tile([33, FQ], f32, tag="dpk", name=f"dpk{qh}{p}")
            nc.vector.memset(dpk[:, :], 1.0)
            nc.vector.tensor_copy(dpk[0:1, :], ops[0][DK:DK + 1, :])
            nc.vector.tensor_copy(dpk[32:33, :], ops[1][DK:DK + 1, :])
            nx = small.tile([33, FQ], f32, tag="nx", name=f"nx{qh}{p}")
            nc.vector.tensor_scalar(
                out=nx.bitcast(i32)[:, :], in0=dpk.bitcast(i32)[:, :],
                scalar1=-1, scalar2=None, op0=mybir.AluOpType.bitwise_xor)
            ya = small.tile([33, FQ], f32, tag="ya", name=f"ya{qh}{p}")
            nc.vector.tensor_scalar_mul(ya[:, :], nx[:, :], C0)
            tt = small.tile([33, FQ], f32, tag="tt", name=f"tt{qh}{p}")
            nc.vector.tensor_mul(tt[:, :], dpk[:, :], ya[:, :])
            nc.vector.tensor_scalar(
                out=tt[:, :], in0=tt[:, :], scalar1=-1.0, scalar2=C1,
                op0=mybir.AluOpType.mult, op1=mybir.AluOpType.add)
            yb = small.tile([33, FQ], f32, tag="yb", name=f"yb{qh}{p}")
            nc.vector.tensor_mul(yb[:, :], ya[:, :], tt[:, :])
            nc.vector.tensor_mul(tt[:, :], dpk[:, :], yb[:, :])
            nc.vector.tensor_scalar(
                out=tt[:, :], in0=tt[:, :], scalar1=-1.0, scalar2=C2,
                op0=mybir.AluOpType.mult, op1=mybir.AluOpType.add)
            nc.vector.tensor_mul(ya[:, :], yb[:, :], tt[:, :])
            for s in range(2):
                row = ya[32 * s:32 * s + 1, :]
                bc = ps.tile([DK, FQ], f32, tag="s", name=f"bc{qh}{p}{s}")
                for c in range(2):
                    nc.tensor.matmul(
                        bc[:, c * 512:(c + 1) * 512],
                        ones128[32 * s:32 * s + 1, 0:DK],
                        row[:, c * 512:(c + 1) * 512],
                        start=True, stop=True)
                rbc = small.tile([DK, FQ], f32, tag=f"rbc{s}",
                                 name=f"rbc{qh}{p}{s}")
                nc.vector.tensor_copy(rbc[:, :], bc[:, :])
                nc.vector.tensor_mul(
                    on_sb[p][s * 64:s * 64 + 64, q0:q0 + FQ],
                    ops[s][0:DK, :], rbc[:, :])

        attention(0, 0)
        attention(0, 1)
        q_bf1 = load_half(qT, 1, "q")
        proj_half(wq_bf, q_bf1, qh_sb, bq_sb, 1, "q")
        attention(1, 0)
        attention(1, 1)
        for nt in range(8, 16):
            emit_y(nt)

    nc.compile()
    return nc


_CACHED_NC = None


def _in_maps(q, k, v, attention_mask, Wq, bq, Wk, bk, Wv, bv, Wo, bo,
             m_k, m_v):
    def tile_x(x_b):
        xT = np.asarray(x_b, np.float32).T
        return np.ascontiguousarray(
            xT.reshape(D, 2, FQ).transpose(1, 0, 2))

    def tile_w(wT_slice):
        C = wT_slice.shape[1]
        return np.ascontiguousarray(
            wT_slice.reshape(8, 128, C).transpose(1, 0, 2))

    mk_s = np.asarray(m_k[0], np.float32) * float(np.sqrt(DK))
    mv_s = np.asarray(m_v[0], np.float32) * float(np.sqrt(M))
    WqT = np.asarray(Wq, np.float32).T
    WkT = np.asarray(Wk, np.float32).T
    WvT = np.asarray(Wv, np.float32).T
    WoT = np.asarray(Wo, np.float32).T
    maps = []
    for core in range(8):
        b, part = divmod(core, 4)
        c0 = part * CPC
        msk = np.asarray(attention_mask[b, 0, 0])
        mbias = np.zeros(JT * 128, np.float32)
        mbias[:N] = np.where(msk, MASK_BIAS, 0.0)
        maps.append({
            "qT": tile_x(q[b]),
            "kT": tile_x(k[b]),
            "vT": tile_x(v[b]),
            "wqT": tile_w(WqT[:, c0:c0 + CPC]),
            "wkT": tile_w(WkT[:, c0:c0 + CPC]),
            "wvT": tile_w(WvT[:, c0:c0 + CPC]),
            "woT": np.ascontiguousarray(
                WoT[c0:c0 + CPC, :].reshape(2, 128, D).transpose(1, 0, 2)),
            "mkT": np.ascontiguousarray(mk_s.T[c0:c0 + CPC, :]),
            "mv": np.ascontiguousarray(mv_s[:, c0:c0 + CPC]),
            "bqT": np.ascontiguousarray(
                np.asarray(bq, np.float32)[c0:c0 + CPC].reshape(2, 128).T),
            "bkT": np.ascontiguousarray(
                np.asarray(bk, np.float32)[c0:c0 + CPC].reshape(2, 128).T),
            "bv": np.ascontiguousarray(
                np.asarray(bv, np.float32)[c0:c0 + CPC].reshape(1, CPC)),
            "mb": np.ascontiguousarray(mbias.reshape(JT, 128).T),
        })
    return maps


def kernel(q, k, v, attention_mask, Wq, bq, Wk, bk, Wv, bv, Wo, bo,
           m_k, m_v):
    from concourse.bass_utils import run_bass_kernel_spmd
    global _CACHED_NC
    if _CACHED_NC is None:
        _CACHED_NC = build_kernel()
    maps = _in_maps(q, k, v, attention_mask, Wq, bq, Wk, bk, Wv, bv,
                    Wo, bo, m_k, m_v)
    res = run_bass_kernel_spmd(_CACHED_NC, maps, core_ids=list(range(8)))
    out = np.zeros((B, N, D), np.float32)
    for core in range(8):
        out[core // 4] += res.results[core]["out"]
    out += np.asarray(bo, np.float32)[None, None, :]
    return out


# revision 20
# speedup vs baseline: 5.9016x; 2.9425x over previous
import numpy as np

B, N, D, H, DK, M = 2, 2048, 1024, 16, 64, 64
NJ = N + M
JT = 17
CPC = 256
FQ = 1024
MASK_BIAS = -240000.0


def build_kernel(loop_n=None, upto=None):
    from contextlib import ExitStack
    import concourse.bass as bass
    import concourse.tile as tile
    import concourse.mybir as mybir
    from concourse import bacc

    f32 = mybir.dt.float32
    bf16 = mybir.dt.bfloat16
    EXP = mybir.ActivationFunctionType.Exp

    nc = bacc.Bacc(None, target_bir_lowering=False, debug=False)

    qT = nc.declare_dram_parameter("qT", [2, D, FQ], f32, isOutput=False)
    kT = nc.declare_dram_parameter("kT", [2, D, FQ], f32, isOutput=False)
    vT = nc.declare_dram_parameter("vT", [2, D, FQ], f32, isOutput=False)
    wqT = nc.declare_dram_parameter("wqT", [128, 8, CPC], f32, isOutput=False)
    wkT = nc.declare_dram_parameter("wkT", [128, 8, CPC], f32, isOutput=False)
    wvT = nc.declare_dram_parameter("wvT", [128, 8, CPC], f32, isOutput=False)
    woT = nc.declare_dram_parameter("woT", [128, 2, D], f32, isOutput=False)
    mkT = nc.declare_dram_parameter("mkT", [CPC, M], f32, isOutput=False)
    mv = nc.declare_dram_parameter("mv", [M, CPC], f32, isOutput=False)
    bqT = nc.declare_dram_parameter("bqT", [128, 2], f32, isOutput=False)
    bkT = nc.declare_dram_parameter("bkT", [128, 2], f32, isOutput=False)
    bv = nc.declare_dram_parameter("bv", [1, CPC], f32, isOutput=False)
    mb = nc.declare_dram_parameter("mb", [128, JT], f32, isOutput=False)
    out = nc.declare_dram_parameter("out", [N, D], f32, isOutput=True)

    with tile.TileContext(nc) as tc, ExitStack() as ctx:
        if loop_n is not None:
            ctx.enter_context(tc.For_i(0, loop_n, 1))
        consts = ctx.enter_context(tc.tile_pool(name="consts", bufs=1))
        wpool = ctx.enter_context(tc.tile_pool(name="wpool", bufs=1))
        proj = ctx.enter_context(tc.tile_pool(name="proj", bufs=1))
        onrm = ctx.enter_context(tc.tile_pool(name="onrm", bufs=1))
        small = ctx.enter_context(tc.tile_pool(name="small", bufs=1))
        ysb = ctx.enter_context(tc.tile_pool(name="ysb", bufs=2))
        ps = ctx.enter_context(tc.tile_pool(name="ps", bufs=2, space="PSUM"))
        po = ctx.enter_context(tc.tile_pool(name="po", bufs=2, space="PSUM"))

        ones128 = consts.tile([128, 128], f32, tag="ones")
        nc.vector.memset(ones128, 1.0)
        mb_sb = consts.tile([128, JT], f32, tag="mb")
        nc.sync.dma_start(out=mb_sb, in_=mb[:])
        bq_sb = consts.tile([128, 2], f32, tag="bq")
        nc.sync.dma_start(out=bq_sb, in_=bqT[:])
        bk_sb = consts.tile([128, 2], f32, tag="bk")
        nc.sync.dma_start(out=bk_sb, in_=bkT[:])
        bv_sb = consts.tile([1, CPC], f32, tag="bv")
        nc.sync.dma_start(out=bv_sb, in_=bv[:])
        bvb_ps = ps.tile([128, CPC], f32, tag="s")
        nc.tensor.matmul(bvb_ps[:, :], ones128[0:1, :], bv_sb[:, :],
                         start=True, stop=True)
        bvb_sb = consts.tile([128, 4, DK], f32, tag="bvb")
        nc.vector.tensor_copy(
            bvb_sb[:, :, :], bvb_ps.rearrange("p (h d) -> p h d", h=4))

        qh_sb = [proj.tile([128, N], bf16, tag=f"qh{p}", name=f"qh{p}")
                 for p in range(2)]
        kh_sb = [proj.tile([128, NJ], bf16, tag=f"kh{p}", name=f"kh{p}")
                 for p in range(2)]
        vt_sb = [proj.tile([128, 4, DK + 1], bf16, tag=f"vt{j}",
                           name=f"vt{j}") for j in range(JT)]
        on_sb = [onrm.tile([128, N], bf16, tag=f"on{p}", name=f"on{p}")
                 for p in range(2)]

        pha = ExitStack()
        inbf = pha.enter_context(tc.tile_pool(name="inbf", bufs=2))
        stage = pha.enter_context(tc.tile_pool(name="stage", bufs=2))

        def load_weight(dram, tag):
            wf = stage.tile([128, 8, CPC], f32, tag="wstage",
                            name=f"wf_{tag}")
            nc.sync.dma_start(out=wf, in_=dram[:])
            wb = wpool.tile([128, 8, CPC], bf16, tag=tag, name=f"wb_{tag}")
            nc.vector.tensor_copy(wb[:, :, :], wf[:, :, :])
            return wb

        def load_half(dram, nh, nm):
            xb = inbf.tile([128, 8, FQ], bf16, tag="xbf", name=f"xbf_{nm}{nh}")
            for dc in range(8):
                xf = stage.tile([128, FQ], f32, tag="xstage",
                                name=f"xf_{nm}{nh}{dc}", bufs=3)
                nc.sync.dma_start(
                    out=xf, in_=dram[nh, dc * 128:(dc + 1) * 128, :])
                nc.vector.tensor_copy(xb[:, dc, :], xf[:, :])
            return xb

        def proj_half(wb, x_bf, out_sb, b_sb, nh, nm):
            for p in range(2):
                for qc in range(2):
                    pp = ps.tile([128, 512], f32, tag="s",
                                 name=f"pp_{nm}{nh}{p}{qc}")
                    for dc in range(8):
                        nc.tensor.matmul(
                            pp[:, :],
                            wb[:, dc, p * 128:(p + 1) * 128],
                            x_bf[:, dc, qc * 512:(qc + 1) * 512],
                            start=(dc == 0), stop=(dc == 7))
                    o0 = nh * FQ + qc * 512
                    nc.vector.tensor_scalar_add(
                        out_sb[p][:, o0:o0 + 512], pp[:, :], b_sb[:, p:p + 1])

        def vproj_half(wv_bf, v_bf, nh):
            for t in range(8):
                jt = nh * 8 + t
                pp = ps.tile([128, CPC], f32, tag="s", name=f"ppv{jt}")
                for dc in range(8):
                    nc.tensor.matmul(
                        pp[:, :],
                        v_bf[:, dc, t * 128:(t + 1) * 128],
                        wv_bf[:, dc, :],
                        start=(dc == 0), stop=(dc == 7))
                vt = vt_sb[jt]
                nc.vector.tensor_add(
                    vt[:, :, 0:DK],
                    pp.rearrange("p (h d) -> p h d", h=4),
                    bvb_sb[:, :, :])
                nc.vector.memset(vt[:, :, DK:DK + 1], 1.0)

        wq_bf = load_weight(wqT, "wq")
        wk_bf = load_weight(wkT, "wk")
        wv_bf = load_weight(wvT, "wv")
        wo_f = stage.tile([128, 2, D], f32, tag="wstage", name="wo_f")
        nc.sync.dma_start(out=wo_f, in_=woT[:])
        wo_bf = wpool.tile([128, 2, D], bf16, tag="wo", name="wo_bf")
        nc.vector.tensor_copy(wo_bf[:, :, :], wo_f[:, :, :])

        k_bf0 = load_half(kT, 0, "k")
        proj_half(wk_bf, k_bf0, kh_sb, bk_sb, 0, "k")
        k_bf1 = load_half(kT, 1, "k")
        proj_half(wk_bf, k_bf1, kh_sb, bk_sb, 1, "k")
        for p in range(2):
            mkf = stage.tile([128, M], f32, tag="xstage", bufs=3,
                             name=f"mkf{p}")
            nc.sync.dma_start(out=mkf, in_=mkT[p * 128:(p + 1) * 128, :])
            nc.vector.tensor_copy(kh_sb[p][:, N:NJ], mkf[:, :])

        q_bf0 = load_half(qT, 0, "q")
        proj_half(wq_bf, q_bf0, qh_sb, bq_sb, 0, "q")

        v_bf0 = load_half(vT, 0, "v")
        vproj_half(wv_bf, v_bf0, 0)
        v_bf1 = load_half(vT, 1, "v")
        vproj_half(wv_bf, v_bf1, 1)
        mvf = stage.tile([M, CPC], f32, tag="xstage", bufs=3, name="mvf")
        nc.sync.dma_start(out=mvf, in_=mv[:])
        vt = vt_sb[16]
        nc.vector.tensor_copy(
            vt[0:M, :, 0:DK], mvf.rearrange("p (h d) -> p h d", h=4))
        nc.vector.memset(vt[0:M, :, DK:DK + 1], 1.0)

        q_bf1 = load_half(qT, 1, "q")
        proj_half(wq_bf, q_bf1, qh_sb, bq_sb, 1, "q")
        pha.close()
        if upto == "loads":
            nc.sync.dma_start(out=out[0:128, 0:1024],
                              in_=qh_sb[0].bitcast(f32))
            nc.sync.dma_start(out=out[128:256, 0:1024],
                              in_=qh_sb[1].bitcast(f32))
            nc.sync.dma_start(out=out[256:384, 0:1024],
                              in_=kh_sb[0].bitcast(f32)[:, 0:1024])
            nc.sync.dma_start(out=out[384:512, 0:1024],
                              in_=kh_sb[1].bitcast(f32)[:, 0:1024])
            tok = consts.tile([128, 4, DK], bf16, tag="tok", name="tok")
            for jt in range(JT):
                jw = 128 if jt < 16 else M
                nc.vector.tensor_copy(tok[:jw, :, :], vt_sb[jt][:jw, :, 0:DK])
            nc.sync.dma_start(out=out[512:640, 0:128], in_=tok.bitcast(f32))
            nc.compile()
            return nc
        ptp = ctx.enter_context(tc.tile_pool(name="ptp", bufs=38))

        def emit_y(nt):
            yp = ps.tile([128, D], f32, tag="s", name=f"yp{nt}")
            for c in range(2):
                for cc in range(2):
                    nc.tensor.matmul(
                        yp[:, c * 512:(c + 1) * 512],
                        on_sb[cc][:, nt * 128:(nt + 1) * 128],
                        wo_bf[:, cc, c * 512:(c + 1) * 512],
                        start=(cc == 0), stop=(cc == 1))
            yt = ysb.tile([128, D], f32, tag="y", name=f"yt{nt}")
            nc.vector.tensor_copy(yt[:, :], yp[:, :])
            nc.sync.dma_start(out=out[nt * 128:(nt + 1) * 128, :], in_=yt)

        i32 = mybir.dt.int32
        C0, C1, C2 = -0.23549792, 2.0017324, 2.0

        def pv_step(g, ops_g, jt):
            qh, p, pt = g
            jw = 128 if jt < 16 else M
            for s in range(2):
                for c in range(2):
                    nc.tensor.matmul(
                        ops_g[s][:, c * 512:(c + 1) * 512],
                        vt_sb[jt][:jw, 2 * p + s, :],
                        pt[(s, jt)][:jw, c * 512:(c + 1) * 512],
                        start=(jt == 0), stop=(jt == JT - 1))

        def norm(g, ops_g):
            qh, p, pt = g
            q0 = qh * FQ
            dpk = small.tile([33, FQ], f32, tag="dpk", name=f"dpk{qh}{p}")
            nc.vector.memset(dpk[:, :], 1.0)
            nc.vector.tensor_copy(dpk[0:1, :], ops_g[0][DK:DK + 1, :])
            nc.vector.tensor_copy(dpk[32:33, :], ops_g[1][DK:DK + 1, :])
            nx = small.tile([33, FQ], f32, tag="nx", name=f"nx{qh}{p}")
            nc.vector.tensor_scalar(
                out=nx.bitcast(i32)[:, :], in0=dpk.bitcast(i32)[:, :],
                scalar1=-1, scalar2=None, op0=mybir.AluOpType.bitwise_xor)
            ya = small.tile([33, FQ], f32, tag="ya", name=f"ya{qh}{p}")
            nc.vector.tensor_scalar_mul(ya[:, :], nx[:, :], C0)
            tt = small.tile([33, FQ], f32, tag="tt", name=f"tt{qh}{p}")
            nc.vector.tensor_mul(tt[:, :], dpk[:, :], ya[:, :])
            nc.vector.tensor_scalar(
                out=tt[:, :], in0=tt[:, :], scalar1=-1.0, scalar2=C1,
                op0=mybir.AluOpType.mult, op1=mybir.AluOpType.add)
            yb = small.tile([33, FQ], f32, tag="yb", name=f"yb{qh}{p}")
            nc.vector.tensor_mul(yb[:, :], ya[:, :], tt[:, :])
            nc.vector.tensor_mul(tt[:, :], dpk[:, :], yb[:, :])
            nc.vector.tensor_scalar(
                out=tt[:, :], in0=tt[:, :], scalar1=-1.0, scalar2=C2,
                op0=mybir.AluOpType.mult, op1=mybir.AluOpType.add)
            nc.vector.tensor_mul(ya[:, :], yb[:, :], tt[:, :])
            for s in range(2):
                row = ya[32 * s:32 * s + 1, :]
                bc = ps.tile([DK, FQ], f32, tag="s", name=f"bc{qh}{p}{s}")
                for c in range(2):
                    nc.tensor.matmul(
                        bc[:, c * 512:(c + 1) * 512],
                        ones128[32 * s:32 * s + 1, 0:DK],
                        row[:, c * 512:(c + 1) * 512],
                        start=True, stop=True)
                rbc = small.tile([DK, FQ], f32, tag=f"rbc{s}",
                                 name=f"rbc{qh}{p}{s}")
                nc.vector.tensor_copy(rbc[:, :], bc[:, :])
                nc.vector.tensor_mul(
                    on_sb[p][s * 64:s * 64 + 64, q0:q0 + FQ],
                    ops_g[s][0:DK, :], rbc[:, :])

        prev = None
        prev_ops = None
        for qh, p in ((0, 0), (0, 1), (1, 0), (1, 1)):
            q0 = qh * FQ
            pt = {}
            ops_prev = None
            if prev is not None:
                ops_prev = [po.tile([DK + 1, FQ], f32, tag="o",
                                    name=f"op{prev[0]}{prev[1]}{s}")
                            for s in range(2)]
            for jt in range(JT):
                jw = 128 if jt < 16 else M
                for s in range(2):
                    sp = ps.tile([128, FQ], f32, tag="s",
                                 name=f"sp{qh}{p}{jt}{s}")
                    for c in range(2):
                        nc.tensor.matmul(
                            sp[:jw, c * 512:(c + 1) * 512],
                            kh_sb[p][s * 64:s * 64 + 64,
                                     jt * 128:jt * 128 + jw],
                            qh_sb[p][s * 64:s * 64 + 64,
                                     q0 + c * 512:q0 + (c + 1) * 512],
                            start=True, stop=True)
                    pe = ptp.tile([128, FQ], bf16, tag="pt",
                                  name=f"pt{qh}{p}{jt}{s}")
                    nc.scalar.activation(
                        out=pe[:jw, :], in_=sp[:jw, :], func=EXP,
                        bias=mb_sb[:jw, jt:jt + 1], scale=0.125)
                    pt[(s, jt)] = pe
                if prev is not None:
                    pv_step(prev, ops_prev, jt)
                if (qh, p) == (1, 1) and jt % 2 == 1:
                    emit_y(jt // 2)
            if prev is not None:
                norm(prev, ops_prev)
            prev = (qh, p, pt)
        ops_last = [po.tile([DK + 1, FQ], f32, tag="o", name=f"opL{s}")
                    for s in range(2)]
        for jt in range(JT):
            pv_step(prev, ops_last, jt)
        norm(prev, ops_last)
        for nt in range(8, 16):
            emit_y(nt)

    nc.compile()
    return nc


_CACHED_NC = None


def _in_maps(q, k, v, attention_mask, Wq, bq, Wk, bk, Wv, bv, Wo, bo,
             m_k, m_v):
    def tile_x(x_b):
        xT = np.asarray(x_b, np.float32).T
        return np.ascontiguousarray(
            xT.reshape(D, 2, FQ).transpose(1, 0, 2))

    def tile_w(wT_slice):
        C = wT_slice.shape[1]
        return np.ascontiguousarray(
            wT_slice.reshape(8, 128, C).transpose(1, 0, 2))

    mk_s = np.asarray(m_k[0], np.float32) * float(np.sqrt(DK))
    mv_s = np.asarray(m_v[0], np.float32) * float(np.sqrt(M))
    WqT = np.asarray(Wq, np.float32).T
    WkT = np.asarray(Wk, np.float32).T
    WvT = np.asarray(Wv, np.float32).T
    WoT = np.asarray(Wo, np.float32).T
    maps = []
    for core in range(8):
        b, part = divmod(core, 4)
        c0 = part * CPC
        msk = np.asarray(attention_mask[b, 0, 0])
        mbias = np.zeros(JT * 128, np.float32)
        mbias[:N] = np.where(msk, MASK_BIAS, 0.0)
        maps.append({
            "qT": tile_x(q[b]),
            "kT": tile_x(k[b]),
            "vT": tile_x(v[b]),
            "wqT": tile_w(WqT[:, c0:c0 + CPC]),
            "wkT": tile_w(WkT[:, c0:c0 + CPC]),
            "wvT": tile_w(WvT[:, c0:c0 + CPC]),
            "woT": np.ascontiguousarray(
                WoT[c0:c0 + CPC, :].reshape(2, 128, D).transpose(1, 0, 2)),
            "mkT": np.ascontiguousarray(mk_s.T[c0:c0 + CPC, :]),
            "mv": np.ascontiguousarray(mv_s[:, c0:c0 + CPC]),
            "bqT": np.ascontiguousarray(
                np.asarray(bq, np.float32)[c0:c0 + CPC].reshape(2, 128).T),
            "bkT": np.ascontiguousarray(
                np.asarray(bk, np.float32)[c0:c0 + CPC].reshape(2, 128).T),
            "bv": np.ascontiguousarray(
                np.asarray(bv, np.float32)[c0:c0 + CPC].reshape(1, CPC)),
            "mb": np.ascontiguousarray(mbias.reshape(JT, 128).T),
        })
    return maps


def kernel(q, k, v, attention_mask, Wq, bq, Wk, bk, Wv, bv, Wo, bo,
           m_k, m_v):
    from concourse.bass_utils import run_bass_kernel_spmd
    global _CACHED_NC
    if _CACHED_NC is None:
        _CACHED_NC = build_kernel()
    maps = _in_maps(q, k, v, attention_mask, Wq, bq, Wk, bk, Wv, bv,
                    Wo, bo, m_k, m_v)
    res = run_bass_kernel_spmd(_CACHED_NC, maps, core_ids=list(range(8)))
    out = np.zeros((B, N, D), np.float32)
    for core in range(8):
        out[core // 4] += res.results[core]["out"]
    out += np.asarray(bo, np.float32)[None, None, :]
    return out
